# revision 6
# baseline (speedup 1.0000x reference)
"""Paged GQA decode attention (sparse_attention) on 8 TRN2 NeuronCores.

Sharding: tensor-parallel by KV head (8 heads -> 8 cores). Each core gets its
head's slice of the KV pool, pre-merged on host into single bf16 rows
[khi(128) | vhi(128)] (512 B) so that ONE natural dma_gather per (group,
pool-half) fetches both K and V for a token: half the HBM bytes and half the
descriptors of a hi/lo split scheme, at the 512 B descriptor size that
avoids the sub-512B DMA latency penalty.

Per core dataflow (fully specialized at build time on the actual seq_lens /
pool-half split, identical across cores):
  gather: kv[tok, 0:128]=K, kv[tok, 128:256]=V  (natural layout, tok on
          partitions, one 512 B descriptor per token)
  K^T:    per 128-token slot, PE transpose K chunk -> PSUM (bf16), batched
          8 slots/bank; PSUM->SBUF copies on DVE (its only duty, so the
          cross-group pipeline never stalls on softmax)
  QK:     scores^T[tok,4] = ktT @ (qhi|qlo)   (2 matmuls, 4-col streams)
  exp:    one ACT Exp per group bank -> p^T in SBUF directly as bf16
  PV:     o^T[d,4] accum with V-natural stationary, p as 4-col moving
  sums:   ones-vector matmul -> per-slot partial sums; final reduction and
          softmax normalization happen on host.
Padded tail positions index a zeroed spare pool row (K=0 -> exp(0)=1,
V=0 -> no PV contribution); the host subtracts the known pad count from
each softmax sum. If a pool half has no unreferenced row to zero (only
possible when every slot of that half is referenced), that half falls back
to mask-column multiplies on DVE.
"""

import os

import numpy as np
import ml_dtypes

import concourse.bacc as bacc
import concourse.bass as bass
import concourse.mybir as mybir
import concourse.tile as tile
from concourse.bass_utils import run_bass_kernel_spmd

B, S, HQ, HKV, D = 32, 2048, 32, 8, 128
G = HQ // HKV
POOL = B * S
HALF = POOL // 2
SCALE = D ** -0.5
NCORES = 8
# variable group sizes: tiny first group (fast pipeline fill) and tiny last
# group (short drain tail); big middle groups amortize per-gather overheads
GSIZES = (1, 2, 4, 4, 4, 4, 4, 4, 4, 1)
GROUPS = len(GSIZES)
GOFF = tuple(int(np.sum(GSIZES[:g])) for g in range(GROUPS + 1))
TB = 8             # K^T transpose slots per PSUM bank / copy batch

BF16 = ml_dtypes.bfloat16

_prog_cache: dict = {}
LAST_RESULT = None  # test.py introspection (exec time etc.)


def _pad128(n):
    return (n + 127) // 128 * 128


def _layout(meta, mask_halves):
    """meta[g][h][j] = valid token count of request j in half h of group g.

    mask_halves[h]: True when half h needs DVE mask-column fallback (no
    spare zero row); otherwise pads read the zero row and masks are skipped.

    Returns bookkeeping: per group: merged slot table (lo-half slots then
    hi-half slots), per-request slot lists + sum ranges + mask column ids,
    plus each group-half's column offset into the merged idx tensor.
    """
    info = []
    mask_cols = []  # list of (g, h, j, valid_in_last_slot) -> mask col id
    icol = 0  # running column offset into the merged idx tensor
    for g in range(GROUPS):
        lo_secs, hi_secs = meta[g]
        halves = []
        for h, secs in enumerate((lo_secs, hi_secs)):
            starts, slot_cnt = [], []
            pos = 0
            for j in range(GSIZES[g]):
                starts.append(pos // 128)
                slot_cnt.append(_pad128(secs[j]) // 128)
                pos += _pad128(secs[j])
            halves.append(dict(n=pos, slots=pos // 128, ioff=icol,
                               starts=starts, slot_cnt=slot_cnt, secs=secs))
            icol += pos // 16
        n_lo_slots = halves[0]["slots"]
        nslots = n_lo_slots + halves[1]["slots"]
        # per request: global slot ids, contiguous ranges, masked tail slots
        req_slots, req_ranges, req_masks = [], [], []
        for j in range(GSIZES[g]):
            slots, ranges, masks = [], [], []
            for h in (0, 1):
                hh = halves[h]
                base = 0 if h == 0 else n_lo_slots
                s0, cnt = hh["starts"][j], hh["slot_cnt"][j]
                if cnt:
                    ranges.append((base + s0, cnt))
                    slots.extend(range(base + s0, base + s0 + cnt))
                    tail = hh["secs"][j] % 128
                    if tail and mask_halves[h]:  # fallback mask col
                        mid = len(mask_cols)
                        mask_cols.append((g, h, j, tail))
                        masks.append((base + s0 + cnt - 1, mid))
            req_slots.append(slots)
            req_ranges.append(ranges)
            req_masks.append(masks)
        info.append(dict(halves=halves, nslots=nslots,
                         req_slots=req_slots, req_ranges=req_ranges,
                         req_masks=req_masks))
    return info, mask_cols, icol


def _build_program(meta, mask_halves):
    info, mask_cols, idx_w = _layout(meta, mask_halves)
    n_mask = max(1, len(mask_cols))
    dt = mybir.dt
    nc = bacc.Bacc(trn_type="TRN2")

    kv_il = nc.dram_tensor("kv_il", [POOL, 256], dt.bfloat16, kind="ExternalInput")
    qhiT = nc.dram_tensor("qhiT", [128, 128], dt.bfloat16, kind="ExternalInput")
    qloT = nc.dram_tensor("qloT", [128, 128], dt.bfloat16, kind="ExternalInput")
    identd = nc.dram_tensor("identd", [128, 128], dt.bfloat16, kind="ExternalInput")
    maskc_d = nc.dram_tensor("maskc", [128, n_mask], dt.float32, kind="ExternalInput")
    idx_w = max(1, idx_w)
    idx_d = nc.dram_tensor("idx_all", [128, idx_w], dt.int16, kind="ExternalInput")
    o_dram = nc.dram_tensor("o_un", [128, B * G], dt.float32, kind="ExternalOutput")
    s_dram = nc.dram_tensor("sums", [GROUPS, 512], dt.float32, kind="ExternalOutput")

    with tile.TileContext(nc) as tc:
        with (
            tc.tile_pool(name="const", bufs=1) as cpool,
            tc.tile_pool(name="kv", bufs=3) as kvp,
            tc.tile_pool(name="ktT", bufs=2) as ktp,
            tc.tile_pool(name="pt", bufs=2) as ptp,
            tc.tile_pool(name="stg", bufs=2) as stgp,
            tc.tile_pool(name="ps_kt", bufs=2, space="PSUM") as pskt,
            tc.tile_pool(name="ps_sc", bufs=2, space="PSUM") as pssc,
            tc.tile_pool(name="ps_pv", bufs=2, space="PSUM") as pspv,
            tc.tile_pool(name="ps_sm", bufs=2, space="PSUM") as pssm,
        ):
            qhi_t = cpool.tile([128, 128], dt.bfloat16, tag="qhi")
            qlo_t = cpool.tile([128, 128], dt.bfloat16, tag="qlo")
            ident_t = cpool.tile([128, 128], dt.bfloat16, tag="ident")
            ones_t = cpool.tile([128, 1], dt.bfloat16, tag="ones")
            mask_t = cpool.tile([128, n_mask], dt.float32, tag="maskc")
            idx_t = cpool.tile([128, idx_w], dt.int16, tag="idxall")
            # group-0 idx first (unblocks gather 0), then the small constant
            # uploads (ident gates the first PE transpose!), then the rest
            def _idx_dma(g):
                for h in (0, 1):
                    hh = info[g]["halves"][h]
                    n = hh["n"]
                    if n == 0:
                        continue
                    i0 = hh["ioff"]
                    nc.sync.dma_start(out=idx_t[:, i0:i0 + n // 16],
                                      in_=idx_d[:, i0:i0 + n // 16])
            _idx_dma(0)
            nc.sync.dma_start(out=ident_t[:], in_=identd[:])
            nc.sync.dma_start(out=qhi_t[:], in_=qhiT[:])
            nc.sync.dma_start(out=qlo_t[:], in_=qloT[:])
            nc.sync.dma_start(out=mask_t[:], in_=maskc_d[:])
            for g in range(1, GROUPS):
                _idx_dma(g)
            nc.vector.memset(ones_t[:], 1.0)

            for g in range(GROUPS):
                gi = info[g]
                nslots = gi["nslots"]
                ncols = 4 * nslots
                OC = G * GSIZES[g]
                ob = G * GOFF[g]
                if nslots == 0:
                    # all requests in this group are empty (degenerate input)
                    z = stgp.tile([128, OC], dt.float32, tag="ostg")
                    nc.vector.memset(z[:], 0.0)
                    nc.sync.dma_start(out=o_dram[:, ob:ob + OC], in_=z[:])
                    continue
                # --- one merged K|V gather per pool half ------------------
                kvt = kvp.tile([128, nslots, 256], dt.bfloat16, tag="kv")
                n_lo_slots = gi["halves"][0]["slots"]
                for h in (0, 1):
                    n = gi["halves"][h]["n"]
                    if n == 0:
                        continue
                    ioff = gi["halves"][h]["ioff"]
                    it = idx_t[:, ioff:ioff + n // 16]
                    src = kv_il[0:HALF, :] if h == 0 else kv_il[HALF:POOL, :]
                    sb = 0 if h == 0 else n_lo_slots
                    nc.gpsimd.dma_gather(
                        out_ap=kvt[:, sb:sb + n // 128, :], in_ap=src,
                        idxs_ap=it, num_idxs=n, num_idxs_reg=n, elem_size=256,
                        transpose=False, single_packet=False)

                # --- K^T: PE transpose batches + PSUM->SBUF copies ---------
                # copies go 2:1 DVE:ACT to balance engine busy
                ktT = ktp.tile([128, nslots * 128], dt.bfloat16, tag="ktT")
                for bi, s0 in enumerate(range(0, nslots, TB)):
                    nb = min(TB, nslots - s0)
                    kt_ps = pskt.tile([128, TB * 128], dt.bfloat16, tag="ktps")
                    for i in range(nb):
                        nc.tensor.transpose(kt_ps[:, 128 * i:128 * (i + 1)],
                                            kvt[:, s0 + i, 0:128], ident_t[:])
                    dst = ktT[:, 128 * s0:128 * (s0 + nb)]
                    if bi % 3 == 2:
                        nc.scalar.activation(dst, kt_ps[:, 0:128 * nb],
                                             mybir.ActivationFunctionType.Copy)
                    else:
                        nc.vector.tensor_copy(out=dst, in_=kt_ps[:, 0:128 * nb])

                # --- QK: scores^T into one PSUM bank ----------------------
                sc = pssc.tile([128, ncols], dt.float32, tag="sc")
                for s in range(nslots):
                    h = 0 if s < n_lo_slots else 1
                    loc = s if h == 0 else s - n_lo_slots
                    hh = gi["halves"][h]
                    j = max(jj for jj in range(GSIZES[g])
                            if hh["starts"][jj] <= loc)
                    b = GOFF[g] + j
                    kT = ktT[:, 128 * s:128 * (s + 1)]
                    out = sc[:, 4 * s:4 * s + 4]
                    nc.tensor.matmul(out, kT, qhi_t[:, 4 * b:4 * b + 4],
                                     start=True, stop=False)
                    nc.tensor.matmul(out, kT, qlo_t[:, 4 * b:4 * b + 4],
                                     start=False, stop=True)

                # --- softmax numerator, straight to bf16 (scores are O(1))
                pt = ptp.tile([128, ncols], dt.bfloat16, tag="pt")
                nc.scalar.activation(pt[:], sc[:],
                                     mybir.ActivationFunctionType.Exp)
                # mask fallback: zero padded tails lacking a spare zero row
                for j in range(GSIZES[g]):
                    for (gslot, mid) in gi["req_masks"][j]:
                        cc = 4 * gslot
                        nc.vector.tensor_scalar_mul(
                            out=pt[:, cc:cc + 4], in0=pt[:, cc:cc + 4],
                            scalar1=mask_t[:, mid:mid + 1])

                # --- PV (o^T accum, V-natural stationary) + sums ----------
                pv = pspv.tile([128, OC], dt.float32, tag="pv")
                sm = pssm.tile([1, 512], dt.float32, tag="sm")
                for j in range(GSIZES[g]):
                    slots = gi["req_slots"][j]
                    oc = G * j
                    if not slots:
                        nc.vector.memset(pv[:, oc:oc + G], 0.0)
                        continue
                    last = len(slots) - 1
                    for si, s in enumerate(slots):
                        nc.tensor.matmul(pv[:, oc:oc + G], kvt[:, s, 128:256],
                                         pt[:, 4 * s:4 * s + 4],
                                         start=(si == 0), stop=(si == last))
                    for (s0, cnt) in gi["req_ranges"][j]:
                        nc.tensor.matmul(sm[0:1, 4 * s0:4 * (s0 + cnt)],
                                         ones_t[:, 0:1],
                                         pt[:, 4 * s0:4 * (s0 + cnt)],
                                         start=True, stop=True)

                ostg = stgp.tile([128, OC], dt.float32, tag="ostg")
                sstg = stgp.tile([1, 512], dt.float32, tag="sstg")
                nc.scalar.activation(ostg[:], pv[:],
                                     mybir.ActivationFunctionType.Copy)
                nc.scalar.activation(sstg[0:1, 0:ncols], sm[0:1, 0:ncols],
                                     mybir.ActivationFunctionType.Copy)
                nc.sync.dma_start(out=o_dram[:, ob:ob + OC], in_=ostg[:])
                nc.sync.dma_start(out=s_dram[g:g + 1, 0:ncols],
                                  in_=sstg[0:1, 0:ncols])

    nc.compile()
    return nc, info, mask_cols


def prepare(inputs):
    q = np.asarray(inputs["q"], np.float32)
    k = np.asarray(inputs["k"], np.float32)
    v = np.asarray(inputs["v"], np.float32)
    k_buffer = np.asarray(inputs["k_buffer"], np.float32)
    v_buffer = np.asarray(inputs["v_buffer"], np.float32)
    req_to_token = np.asarray(inputs["req_to_token"])
    req_pool_indices = np.asarray(inputs["req_pool_indices"])
    seq_lens = np.asarray(inputs["seq_lens"]).astype(np.int64)
    out_cache_loc = np.asarray(inputs["out_cache_loc"]).astype(np.int64)

    # store_kv_cache scatter (tiny: 32 rows) + per-request token lists
    kb = k_buffer.copy()
    vb = v_buffer.copy()
    kb[out_cache_loc] = k.reshape(B, HKV, D)
    vb[out_cache_loc] = v.reshape(B, HKV, D)
    tok = req_to_token[req_pool_indices]

    # spare (unreferenced) pool row per half -> zeroed pad target
    referenced = np.zeros(POOL, bool)
    for b in range(B):
        referenced[tok[b, :seq_lens[b]]] = True
    free_lo = np.flatnonzero(~referenced[:HALF])
    free_hi = np.flatnonzero(~referenced[HALF:])
    zero_row = [int(free_lo[0]) if len(free_lo) else -1,
                int(free_hi[0]) + HALF if len(free_hi) else -1]
    mask_halves = (zero_row[0] < 0, zero_row[1] < 0)

    # group 0 (size 1): smallest request; last group (size 1): next-smallest;
    # group 1 (size 2): next two; the rest biggest-first into middle groups
    asc = list(np.argsort(seq_lens, kind="stable"))
    mid = asc[4:][::-1]
    order = np.array([asc[0]] + asc[2:4] + mid + [asc[1]], dtype=np.int64)

    meta = []
    idx_blocks = []
    for g in range(GROUPS):
        lo_secs, hi_secs = [], []
        for h in (0, 1):
            parts = []
            secs = lo_secs if h == 0 else hi_secs
            pad_idx = zero_row[h] - (0 if h == 0 else HALF)
            if pad_idx < 0:
                pad_idx = 0  # mask fallback half: any valid row
            for j in range(GSIZES[g]):
                b = int(order[GOFF[g] + j])
                t = tok[b, :seq_lens[b]].astype(np.int64)
                tl = t[t < HALF] if h == 0 else t[t >= HALF] - HALF
                secs.append(len(tl))
                arr = np.full(_pad128(len(tl)), pad_idx, np.int64)
                arr[:len(tl)] = tl
                parts.append(arr)
            full = np.concatenate(parts)
            if len(full):
                # [16, n/16] wrap, replicated into all 8 GPSIMD-core stripes
                idx_blocks.append(
                    np.tile(full.astype(np.int16).reshape(-1, 16).T, (8, 1)))
        meta.append((tuple(lo_secs), tuple(hi_secs)))
    meta = tuple(meta)
    if idx_blocks:
        idx_all = np.ascontiguousarray(np.concatenate(idx_blocks, axis=1))
    else:
        idx_all = np.zeros((128, 1), np.int16)

    key = (meta, mask_halves)
    if key not in _prog_cache:
        _prog_cache[key] = _build_program(meta, mask_halves)
    nc, info, mask_cols = _prog_cache[key]

    maskc = np.ones((128, max(1, len(mask_cols))), np.float32)
    for mid, (_, _, _, tail) in enumerate(mask_cols):
        maskc[:, mid] = (np.arange(128) < tail).astype(np.float32)

    ident = np.eye(128, dtype=BF16)
    in_maps = []
    for c in range(NCORES):
        k_hi = kb[:, c, :].astype(BF16)
        v_hi = vb[:, c, :].astype(BF16)
        qc = (q.reshape(B, HKV, G, D)[order, c] * SCALE).reshape(B * G, D)
        qT = np.ascontiguousarray(qc.T)
        q_hi = qT.astype(BF16)
        q_lo = (qT - q_hi.astype(np.float32)).astype(BF16)
        kv_core = np.concatenate([k_hi, v_hi], axis=1)
        for zr in zero_row:
            if zr >= 0:
                kv_core[zr] = 0
        im = {
            "kv_il": np.ascontiguousarray(kv_core),
            "qhiT": np.ascontiguousarray(q_hi),
            "qloT": np.ascontiguousarray(q_lo),
            "identd": ident,
            "maskc": maskc,
            "idx_all": idx_all,
        }
        in_maps.append(im)
    return nc, info, in_maps, order, mask_halves


def postprocess(results, info, order, mask_halves, cores=None):
    out = np.zeros((B, HQ, D), np.float32)
    for c in (cores if cores is not None else range(NCORES)):
        o_un = results[c]["o_un"]  # [128 d, B*G] o^T columns
        sums = results[c]["sums"]
        for g in range(GROUPS):
            gi = info[g]
            for j in range(GSIZES[g]):
                b = int(order[GOFF[g] + j])
                stot = np.zeros(G, np.float64)
                npad = 0
                for h in (0, 1):
                    sec = gi["halves"][h]["secs"][j]
                    if not mask_halves[h]:
                        npad += _pad128(sec) - sec
                for (s0, cnt) in gi["req_ranges"][j]:
                    seg = sums[g, 4 * s0:4 * (s0 + cnt)].astype(np.float64)
                    stot += seg.reshape(cnt, G).sum(axis=0)
                stot -= npad  # zero-row pads contribute exp(0)=1 each
                oc0 = G * (GOFF[g] + j)
                ov = o_un[:, oc0:oc0 + G]  # [128 d, G]
                with np.errstate(divide="ignore", invalid="ignore"):
                    out[b, c * G:(c + 1) * G, :] = (ov / stot[None, :]).T
    return out.reshape(B, HQ * D).astype(np.float32)


def kernel(**inputs):
    global LAST_RESULT
    nc, info, in_maps, order, mask_halves = prepare(inputs)
    res = run_bass_kernel_spmd(nc, in_maps, core_ids=list(range(NCORES)),
                               trace=False)
    LAST_RESULT = res
    return postprocess(res.results, info, order, mask_halves)


# revision 8
# speedup vs baseline: 1.0244x; 1.0244x over previous
"""Paged GQA decode attention (sparse_attention) on 8 TRN2 NeuronCores.

Sharding: tensor-parallel by KV head (8 heads -> 8 cores). Each core gets its
head's slice of the KV pool, pre-merged on host into single bf16 rows
[khi(128) | vhi(128)] (512 B) so that ONE natural dma_gather per (group,
pool-half) fetches both K and V for a token: half the HBM bytes and half the
descriptors of a hi/lo split scheme, at the 512 B descriptor size that
avoids the sub-512B DMA latency penalty.

Per core dataflow (fully specialized at build time on the actual seq_lens /
pool-half split, identical across cores):
  gather: kv[tok, 0:128]=K, kv[tok, 128:256]=V  (natural layout, tok on
          partitions, one 512 B descriptor per token)
  K^T:    per 128-token slot, PE transpose K chunk -> PSUM (bf16), batched
          8 slots/bank; PSUM->SBUF copies on DVE (its only duty, so the
          cross-group pipeline never stalls on softmax)
  QK:     scores^T[tok,4] = ktT @ (qhi|qlo)   (2 matmuls, 4-col streams)
  exp:    one ACT Exp per group bank -> p^T in SBUF directly as bf16
  PV:     o^T[d,4] accum with V-natural stationary, p as 4-col moving
  sums:   ones-vector matmul -> per-slot partial sums; final reduction and
          softmax normalization happen on host.
Padded tail positions index a zeroed spare pool row (K=0 -> exp(0)=1,
V=0 -> no PV contribution); the host subtracts the known pad count from
each softmax sum. If a pool half has no unreferenced row to zero (only
possible when every slot of that half is referenced), that half falls back
to mask-column multiplies on DVE.
"""

import os

import numpy as np
import ml_dtypes

import concourse.bacc as bacc
import concourse.bass as bass
import concourse.mybir as mybir
import concourse.tile as tile
from concourse.bass_utils import run_bass_kernel_spmd

B, S, HQ, HKV, D = 32, 2048, 32, 8, 128
G = HQ // HKV
POOL = B * S
HALF = POOL // 2
SCALE = D ** -0.5
NCORES = 8
# variable group sizes: small last group (short drain tail); big first and
# middle groups keep the gather pipeline busy and amortize per-gather costs
GSIZES = (4, 4, 4, 4, 4, 4, 4, 3, 1)
GROUPS = len(GSIZES)
GOFF = tuple(int(np.sum(GSIZES[:g])) for g in range(GROUPS + 1))
TB = 8             # K^T transpose slots per PSUM bank / copy batch

BF16 = ml_dtypes.bfloat16

_prog_cache: dict = {}
LAST_RESULT = None  # test.py introspection (exec time etc.)


def _pad128(n):
    return (n + 127) // 128 * 128


def _layout(meta, mask_halves):
    """meta[g][h][j] = valid token count of request j in half h of group g.

    mask_halves[h]: True when half h needs DVE mask-column fallback (no
    spare zero row); otherwise pads read the zero row and masks are skipped.

    Returns bookkeeping: per group: merged slot table (lo-half slots then
    hi-half slots), per-request slot lists + sum ranges + mask column ids,
    plus each group-half's column offset into the merged idx tensor.
    """
    info = []
    mask_cols = []  # list of (g, h, j, valid_in_last_slot) -> mask col id
    icol = 0  # running column offset into the merged idx tensor
    for g in range(GROUPS):
        lo_secs, hi_secs = meta[g]
        halves = []
        for h, secs in enumerate((lo_secs, hi_secs)):
            starts, slot_cnt = [], []
            pos = 0
            for j in range(GSIZES[g]):
                starts.append(pos // 128)
                slot_cnt.append(_pad128(secs[j]) // 128)
                pos += _pad128(secs[j])
            halves.append(dict(n=pos, slots=pos // 128, ioff=icol,
                               starts=starts, slot_cnt=slot_cnt, secs=secs))
            icol += pos // 16
        n_lo_slots = halves[0]["slots"]
        nslots = n_lo_slots + halves[1]["slots"]
        # per request: global slot ids, contiguous ranges, masked tail slots
        req_slots, req_ranges, req_masks = [], [], []
        for j in range(GSIZES[g]):
            slots, ranges, masks = [], [], []
            for h in (0, 1):
                hh = halves[h]
                base = 0 if h == 0 else n_lo_slots
                s0, cnt = hh["starts"][j], hh["slot_cnt"][j]
                if cnt:
                    ranges.append((base + s0, cnt))
                    slots.extend(range(base + s0, base + s0 + cnt))
                    tail = hh["secs"][j] % 128
                    if tail and mask_halves[h]:  # fallback mask col
                        mid = len(mask_cols)
                        mask_cols.append((g, h, j, tail))
                        masks.append((base + s0 + cnt - 1, mid))
            req_slots.append(slots)
            req_ranges.append(ranges)
            req_masks.append(masks)
        info.append(dict(halves=halves, nslots=nslots,
                         req_slots=req_slots, req_ranges=req_ranges,
                         req_masks=req_masks))
    return info, mask_cols, icol


def _build_program(meta, mask_halves):
    info, mask_cols, idx_w = _layout(meta, mask_halves)
    n_mask = max(1, len(mask_cols))
    dt = mybir.dt
    nc = bacc.Bacc(trn_type="TRN2")

    kv_il = nc.dram_tensor("kv_il", [POOL, 256], dt.bfloat16, kind="ExternalInput")
    qhiT = nc.dram_tensor("qhiT", [128, 128], dt.bfloat16, kind="ExternalInput")
    qloT = nc.dram_tensor("qloT", [128, 128], dt.bfloat16, kind="ExternalInput")
    identd = nc.dram_tensor("identd", [128, 128], dt.bfloat16, kind="ExternalInput")
    maskc_d = nc.dram_tensor("maskc", [128, n_mask], dt.float32, kind="ExternalInput")
    idx_w = max(1, idx_w)
    idx_d = nc.dram_tensor("idx_all", [128, idx_w], dt.int16, kind="ExternalInput")
    o_dram = nc.dram_tensor("o_un", [128, B * G], dt.float32, kind="ExternalOutput")
    s_dram = nc.dram_tensor("sums", [GROUPS, 512], dt.float32, kind="ExternalOutput")

    with tile.TileContext(nc) as tc:
        with (
            tc.tile_pool(name="const", bufs=1) as cpool,
            tc.tile_pool(name="kv", bufs=3) as kvp,
            tc.tile_pool(name="ktT", bufs=2) as ktp,
            tc.tile_pool(name="pt", bufs=2) as ptp,
            tc.tile_pool(name="stg", bufs=2) as stgp,
            tc.tile_pool(name="ps_kt", bufs=2, space="PSUM") as pskt,
            tc.tile_pool(name="ps_sc", bufs=2, space="PSUM") as pssc,
            tc.tile_pool(name="ps_pv", bufs=2, space="PSUM") as pspv,
            tc.tile_pool(name="ps_sm", bufs=2, space="PSUM") as pssm,
        ):
            qhi_t = cpool.tile([128, 128], dt.bfloat16, tag="qhi")
            qlo_t = cpool.tile([128, 128], dt.bfloat16, tag="qlo")
            ident_t = cpool.tile([128, 128], dt.bfloat16, tag="ident")
            ones_t = cpool.tile([128, 1], dt.bfloat16, tag="ones")
            mask_t = cpool.tile([128, n_mask], dt.float32, tag="maskc")
            idx_t = cpool.tile([128, idx_w], dt.int16, tag="idxall")
            # group-0 idx first (unblocks gather 0), then the small constant
            # uploads (ident gates the first PE transpose!), then the rest
            def _idx_dma(g):
                for h in (0, 1):
                    hh = info[g]["halves"][h]
                    n = hh["n"]
                    if n == 0:
                        continue
                    i0 = hh["ioff"]
                    nc.sync.dma_start(out=idx_t[:, i0:i0 + n // 16],
                                      in_=idx_d[:, i0:i0 + n // 16])
            _idx_dma(0)
            nc.sync.dma_start(out=ident_t[:], in_=identd[:])
            nc.sync.dma_start(out=qhi_t[:], in_=qhiT[:])
            nc.sync.dma_start(out=qlo_t[:], in_=qloT[:])
            nc.sync.dma_start(out=mask_t[:], in_=maskc_d[:])
            for g in range(1, GROUPS):
                _idx_dma(g)
            nc.vector.memset(ones_t[:], 1.0)

            for g in range(GROUPS):
                gi = info[g]
                nslots = gi["nslots"]
                ncols = 4 * nslots
                OC = G * GSIZES[g]
                ob = G * GOFF[g]
                if nslots == 0:
                    # all requests in this group are empty (degenerate input)
                    z = stgp.tile([128, OC], dt.float32, tag="ostg")
                    nc.vector.memset(z[:], 0.0)
                    nc.sync.dma_start(out=o_dram[:, ob:ob + OC], in_=z[:])
                    continue
                # --- one merged K|V gather per pool half ------------------
                kvt = kvp.tile([128, nslots, 256], dt.bfloat16, tag="kv")
                n_lo_slots = gi["halves"][0]["slots"]
                for h in (0, 1):
                    n = gi["halves"][h]["n"]
                    if n == 0:
                        continue
                    ioff = gi["halves"][h]["ioff"]
                    it = idx_t[:, ioff:ioff + n // 16]
                    src = kv_il[0:HALF, :] if h == 0 else kv_il[HALF:POOL, :]
                    sb = 0 if h == 0 else n_lo_slots
                    nc.gpsimd.dma_gather(
                        out_ap=kvt[:, sb:sb + n // 128, :], in_ap=src,
                        idxs_ap=it, num_idxs=n, num_idxs_reg=n, elem_size=256,
                        transpose=False, single_packet=False)

                # --- K^T: PE transpose batches + PSUM->SBUF copies ---------
                # copies go 2:1 DVE:ACT to balance engine busy
                ktT = ktp.tile([128, nslots * 128], dt.bfloat16, tag="ktT")
                for bi, s0 in enumerate(range(0, nslots, TB)):
                    nb = min(TB, nslots - s0)
                    kt_ps = pskt.tile([128, TB * 128], dt.bfloat16, tag="ktps")
                    for i in range(nb):
                        nc.tensor.transpose(kt_ps[:, 128 * i:128 * (i + 1)],
                                            kvt[:, s0 + i, 0:128], ident_t[:])
                    dst = ktT[:, 128 * s0:128 * (s0 + nb)]
                    if bi % 3 == 2:
                        nc.scalar.activation(dst, kt_ps[:, 0:128 * nb],
                                             mybir.ActivationFunctionType.Copy)
                    else:
                        nc.vector.tensor_copy(out=dst, in_=kt_ps[:, 0:128 * nb])

                # --- QK: scores^T into one PSUM bank ----------------------
                sc = pssc.tile([128, ncols], dt.float32, tag="sc")
                for s in range(nslots):
                    h = 0 if s < n_lo_slots else 1
                    loc = s if h == 0 else s - n_lo_slots
                    hh = gi["halves"][h]
                    j = max(jj for jj in range(GSIZES[g])
                            if hh["starts"][jj] <= loc)
                    b = GOFF[g] + j
                    kT = ktT[:, 128 * s:128 * (s + 1)]
                    out = sc[:, 4 * s:4 * s + 4]
                    nc.tensor.matmul(out, kT, qhi_t[:, 4 * b:4 * b + 4],
                                     start=True, stop=False)
                    nc.tensor.matmul(out, kT, qlo_t[:, 4 * b:4 * b + 4],
                                     start=False, stop=True)

                # --- softmax numerator, straight to bf16 (scores are O(1))
                pt = ptp.tile([128, ncols], dt.bfloat16, tag="pt")
                nc.scalar.activation(pt[:], sc[:],
                                     mybir.ActivationFunctionType.Exp)
                # mask fallback: zero padded tails lacking a spare zero row
                for j in range(GSIZES[g]):
                    for (gslot, mid) in gi["req_masks"][j]:
                        cc = 4 * gslot
                        nc.vector.tensor_scalar_mul(
                            out=pt[:, cc:cc + 4], in0=pt[:, cc:cc + 4],
                            scalar1=mask_t[:, mid:mid + 1])

                # --- PV (o^T accum, V-natural stationary) + sums ----------
                pv = pspv.tile([128, OC], dt.float32, tag="pv")
                sm = pssm.tile([1, 512], dt.float32, tag="sm")
                for j in range(GSIZES[g]):
                    slots = gi["req_slots"][j]
                    oc = G * j
                    if not slots:
                        nc.vector.memset(pv[:, oc:oc + G], 0.0)
                        continue
                    last = len(slots) - 1
                    for si, s in enumerate(slots):
                        nc.tensor.matmul(pv[:, oc:oc + G], kvt[:, s, 128:256],
                                         pt[:, 4 * s:4 * s + 4],
                                         start=(si == 0), stop=(si == last))
                    for (s0, cnt) in gi["req_ranges"][j]:
                        nc.tensor.matmul(sm[0:1, 4 * s0:4 * (s0 + cnt)],
                                         ones_t[:, 0:1],
                                         pt[:, 4 * s0:4 * (s0 + cnt)],
                                         start=True, stop=True)

                ostg = stgp.tile([128, OC], dt.float32, tag="ostg")
                sstg = stgp.tile([1, 512], dt.float32, tag="sstg")
                nc.scalar.activation(ostg[:], pv[:],
                                     mybir.ActivationFunctionType.Copy)
                nc.scalar.activation(sstg[0:1, 0:ncols], sm[0:1, 0:ncols],
                                     mybir.ActivationFunctionType.Copy)
                nc.sync.dma_start(out=o_dram[:, ob:ob + OC], in_=ostg[:])
                nc.sync.dma_start(out=s_dram[g:g + 1, 0:ncols],
                                  in_=sstg[0:1, 0:ncols])

    nc.compile()
    return nc, info, mask_cols


def prepare(inputs):
    q = np.asarray(inputs["q"], np.float32)
    k = np.asarray(inputs["k"], np.float32)
    v = np.asarray(inputs["v"], np.float32)
    k_buffer = np.asarray(inputs["k_buffer"], np.float32)
    v_buffer = np.asarray(inputs["v_buffer"], np.float32)
    req_to_token = np.asarray(inputs["req_to_token"])
    req_pool_indices = np.asarray(inputs["req_pool_indices"])
    seq_lens = np.asarray(inputs["seq_lens"]).astype(np.int64)
    out_cache_loc = np.asarray(inputs["out_cache_loc"]).astype(np.int64)

    # store_kv_cache scatter (tiny: 32 rows) + per-request token lists
    kb = k_buffer.copy()
    vb = v_buffer.copy()
    kb[out_cache_loc] = k.reshape(B, HKV, D)
    vb[out_cache_loc] = v.reshape(B, HKV, D)
    tok = req_to_token[req_pool_indices]

    # spare (unreferenced) pool row per half -> zeroed pad target
    referenced = np.zeros(POOL, bool)
    for b in range(B):
        referenced[tok[b, :seq_lens[b]]] = True
    free_lo = np.flatnonzero(~referenced[:HALF])
    free_hi = np.flatnonzero(~referenced[HALF:])
    zero_row = [int(free_lo[0]) if len(free_lo) else -1,
                int(free_hi[0]) + HALF if len(free_hi) else -1]
    mask_halves = (zero_row[0] < 0, zero_row[1] < 0)

    # group 0: the 4 smallest requests (fast fill); last group: next-smallest
    # alone (short drain); the rest biggest-first into middle groups
    asc = list(np.argsort(seq_lens, kind="stable"))
    mid = asc[5:][::-1]
    order = np.array(asc[:4] + mid + [asc[4]], dtype=np.int64)

    meta = []
    idx_blocks = []
    for g in range(GROUPS):
        lo_secs, hi_secs = [], []
        for h in (0, 1):
            parts = []
            secs = lo_secs if h == 0 else hi_secs
            pad_idx = zero_row[h] - (0 if h == 0 else HALF)
            if pad_idx < 0:
                pad_idx = 0  # mask fallback half: any valid row
            for j in range(GSIZES[g]):
                b = int(order[GOFF[g] + j])
                t = tok[b, :seq_lens[b]].astype(np.int64)
                tl = t[t < HALF] if h == 0 else t[t >= HALF] - HALF
                secs.append(len(tl))
                arr = np.full(_pad128(len(tl)), pad_idx, np.int64)
                arr[:len(tl)] = tl
                parts.append(arr)
            full = np.concatenate(parts)
            if len(full):
                # [16, n/16] wrap, replicated into all 8 GPSIMD-core stripes
                idx_blocks.append(
                    np.tile(full.astype(np.int16).reshape(-1, 16).T, (8, 1)))
        meta.append((tuple(lo_secs), tuple(hi_secs)))
    meta = tuple(meta)
    if idx_blocks:
        idx_all = np.ascontiguousarray(np.concatenate(idx_blocks, axis=1))
    else:
        idx_all = np.zeros((128, 1), np.int16)

    key = (meta, mask_halves)
    if key not in _prog_cache:
        _prog_cache[key] = _build_program(meta, mask_halves)
    nc, info, mask_cols = _prog_cache[key]

    maskc = np.ones((128, max(1, len(mask_cols))), np.float32)
    for mid, (_, _, _, tail) in enumerate(mask_cols):
        maskc[:, mid] = (np.arange(128) < tail).astype(np.float32)

    ident = np.eye(128, dtype=BF16)
    in_maps = []
    for c in range(NCORES):
        k_hi = kb[:, c, :].astype(BF16)
        v_hi = vb[:, c, :].astype(BF16)
        qc = (q.reshape(B, HKV, G, D)[order, c] * SCALE).reshape(B * G, D)
        qT = np.ascontiguousarray(qc.T)
        q_hi = qT.astype(BF16)
        q_lo = (qT - q_hi.astype(np.float32)).astype(BF16)
        kv_core = np.concatenate([k_hi, v_hi], axis=1)
        for zr in zero_row:
            if zr >= 0:
                kv_core[zr] = 0
        im = {
            "kv_il": np.ascontiguousarray(kv_core),
            "qhiT": np.ascontiguousarray(q_hi),
            "qloT": np.ascontiguousarray(q_lo),
            "identd": ident,
            "maskc": maskc,
            "idx_all": idx_all,
        }
        in_maps.append(im)
    return nc, info, in_maps, order, mask_halves


def postprocess(results, info, order, mask_halves, cores=None):
    out = np.zeros((B, HQ, D), np.float32)
    for c in (cores if cores is not None else range(NCORES)):
        o_un = results[c]["o_un"]  # [128 d, B*G] o^T columns
        sums = results[c]["sums"]
        for g in range(GROUPS):
            gi = info[g]
            for j in range(GSIZES[g]):
                b = int(order[GOFF[g] + j])
                stot = np.zeros(G, np.float64)
                npad = 0
                for h in (0, 1):
                    sec = gi["halves"][h]["secs"][j]
                    if not mask_halves[h]:
                        npad += _pad128(sec) - sec
                for (s0, cnt) in gi["req_ranges"][j]:
                    seg = sums[g, 4 * s0:4 * (s0 + cnt)].astype(np.float64)
                    stot += seg.reshape(cnt, G).sum(axis=0)
                stot -= npad  # zero-row pads contribute exp(0)=1 each
                oc0 = G * (GOFF[g] + j)
                ov = o_un[:, oc0:oc0 + G]  # [128 d, G]
                with np.errstate(divide="ignore", invalid="ignore"):
                    out[b, c * G:(c + 1) * G, :] = (ov / stot[None, :]).T
    return out.reshape(B, HQ * D).astype(np.float32)


def kernel(**inputs):
    global LAST_RESULT
    nc, info, in_maps, order, mask_halves = prepare(inputs)
    res = run_bass_kernel_spmd(nc, in_maps, core_ids=list(range(NCORES)),
                               trace=False)
    LAST_RESULT = res
    return postprocess(res.results, info, order, mask_halves)
